# revision 31
# baseline (speedup 1.0000x reference)
"""CRF loss (multi-annotator) Trainium2 kernel.

Problem (hardcoded): scores (8,200,64,32,32) f32, targets (8,200,64) int,
mask (200,64) bool, a_mask (8,64) bool -> scalar f32 loss.

Sharding: one annotator per NeuronCore (8 cores). Host applies a_mask and
sums / B.

Design:
  - Host relayouts scores to bf16 [c2, h2, th2, b32, S, tl16, j16]
    (f = h*16+j from-tag, t = th*16+tl to-tag, b-halves c as 2 independent
    scan chains interleaved for latency hiding). Per-partition-row DRAM
    spans are contiguous, so the stream DMA moves 4KB packets and half
    the bytes of the f32 original.
  - exp on ACT one 8-step block at a time, split into 1K-element pieces
    spread across the block's steps (amortizes the 224-cycle ACT bubble
    and keeps multi-us ops out of the ACT queue).
  - Per scan step and chain: ONE custom DVE op (MUL_CUMSUM_SCALE,
    registered at import: running sum of in0*in1*s0 along the free dim)
    fuses the e*w multiply, the renorm scale, and the cumulative sum.
    Segmented sums over j fall out as differences of page-end samples
    (tiny f32 tensor_sub -> bf16 seg), and ONE bf16 matmul both combines
    the f-halves across partitions AND lands the result directly in the
    state layout (lhsT encodes the th''=h selection). The next step's
    scan reads the state straight from the PSUM tile - no copy hop, so
    the chain is scan -> sub -> matmul -> scan.
  - renorm every RENORM steps: scale = 1/rowsum(pt) (uniform per b since
    pt rows are per-b state copies); fed to the next scan's s0 slot; all
    logs deferred to one end-of-kernel pass via host-built hcum windows.
  - capture-at-cutoff for the valid-prefix mask: each step past the
    earliest cutoff copies pt[END] into its own endbuf column (cheap,
    dependency-free), and ONE masked-sum STT at the end reduces
    endbuf * hit into the per-batch capture.
  - tg energy: dma_gather of 256B blocks from a separate s-major bf16
    copy of scores (chunk-relative int16 indices), then masked-sum STT
    half-chunks on DVE late in the scan, combined with an f32 dup matmul.
"""

import os
import sys

import numpy as np

if os.path.isdir("/opt/trn_rl_repo"):
    sys.path.insert(0, "/opt/trn_rl_repo")

import ml_dtypes  # noqa: E402

import concourse.bass as bass  # noqa: E402
import concourse.tile as tile  # noqa: E402
from concourse import bacc, mybir  # noqa: E402
from concourse.bass_utils import run_bass_kernel_spmd  # noqa: E402

F32 = mybir.dt.float32
BF16 = mybir.dt.bfloat16
I16 = mybir.dt.int16

A, S, B, T = 8, 200, 64, 32
START_TAG, END_TAG = 30, 31
SBLK = 8      # steps per streamed DMA block
GBLK = 16     # steps per dma_gather chunk
RENORM = 8    # renorm period (steps)

BC = 32       # batch elements per chain
NCHAIN = 2

# ---------------------------------------------------------------------------
# Custom DVE op: out[k] = running_sum(in0*in1*s0) (inclusive, whole stream)
# ---------------------------------------------------------------------------


def _register_mul_cumsum():
    import concourse.dve_ops as dve_ops
    from concourse.dve_ops import OPS, DveOp, DveOpSpec
    from concourse.dve_spec import AluOp, Spec, Src0, Src1, C0, lower, scan

    name = "MUL_CUMSUM_SCALE"
    for op in OPS:
        if op.name == name:
            return op

    spec = Spec(
        body=scan(AluOp.ADD, Src0 * Src1 * C0),
        reference=lambda in0, in1, s0: np.cumsum(
            in0.astype(np.float32) * in1 * s0, axis=-1
        ),
    )
    row = dve_ops._CUSTOM_DVE_ROW_BASE + len(OPS)
    shas = {}
    for ver in ("v3", "v4"):
        shas[ver] = DveOpSpec(
            name=name, opcode=row, uops=lower(spec, ver=ver), rd1_en=True
        ).sha(ver)
    op = DveOp(name, spec, subdim=False, uops_sha=shas)
    OPS.append(op)
    dve_ops.CUSTOM_DVE_SPECS[name] = spec
    dve_ops._SUB_OPCODE_FOR_NAME[name] = row
    return op


MUL_CUMSUM_SCALE = _register_mul_cumsum()


def _plan(S):
    """Gather chunk plan: list of (s0, nsteps, idx_col0, out_col0)."""
    chunks = []
    s0 = 0
    idx_col = 0
    out_col = 0
    while s0 < S:
        ns = min(GBLK, S - s0)
        ni = ns * B
        assert ni % 128 == 0
        chunks.append((s0, ns, idx_col, out_col))
        idx_col += ni // 16
        out_col += ni // 128
        s0 += ns
    return chunks, idx_col, out_col


def _n_renorms(S):
    # renorm triggered after steps s = RENORM, 2*RENORM, ... <= S-2
    return max(0, (S - 2) // RENORM)


def build_nc(S=S):
    from contextlib import ExitStack

    chunks, idx_cols, out_blocks = _plan(S)
    NR = _n_renorms(S)
    smin = S // 2 - 1  # earliest possible hit step (lens >= S//2)
    nblk = (S + SBLK - 1) // SBLK
    ROWELEMS = S * 256  # per-partition-row elements in the relayout

    nc = bacc.Bacc("TRN2", target_bir_lowering=False, debug=False, num_devices=8)

    # streamed scan layout: [chain, 128 rows (h,th,b32), S*256]
    sc_d = nc.dram_tensor("sc", [NCHAIN, 128, ROWELEMS], BF16, kind="ExternalInput").ap()
    # s-major bf16 copy for the tg gather
    scg_d = nc.dram_tensor("scg", [S, B, T, T], BF16, kind="ExternalInput").ap()
    gidx_d = nc.dram_tensor("gidx", [128, idx_cols], I16, kind="ExternalInput").ap()
    oh_d = nc.dram_tensor("oh", [128, out_blocks * 128], BF16, kind="ExternalInput").ap()
    hit_d = nc.dram_tensor("hit", [NCHAIN, 128, S], F32, kind="ExternalInput").ap()
    hcum_d = nc.dram_tensor("hcum", [64, NR], F32, kind="ExternalInput").ap()
    duppb_d = nc.dram_tensor("duppb", [128, 128], BF16, kind="ExternalInput").ap()
    dupi_d = nc.dram_tensor("dupi", [128, 128], BF16, kind="ExternalInput").ap()
    dupf_d = nc.dram_tensor("dupf", [128, 128], F32, kind="ExternalInput").ap()
    out_d = nc.dram_tensor("losses", [64, 1], F32, kind="ExternalOutput").ap()

    with tile.TileContext(nc) as tc, ExitStack() as ctx:
        state = ctx.enter_context(tc.tile_pool(name="state", bufs=1))
        blkp = [
            ctx.enter_context(tc.tile_pool(name=f"blk{c}", bufs=3)) for c in range(2)
        ]
        e16p = [
            ctx.enter_context(tc.tile_pool(name=f"e16{c}", bufs=3)) for c in range(2)
        ]
        work = ctx.enter_context(tc.tile_pool(name="work", bufs=4))
        gathp = ctx.enter_context(tc.tile_pool(name="gath", bufs=1))
        psum = ctx.enter_context(tc.tile_pool(name="psum", bufs=3, space="PSUM"))
        psumg = ctx.enter_context(tc.tile_pool(name="psumg", bufs=1, space="PSUM"))

        # ---- persistent state ----
        wsp = [state.tile([128, 16], BF16, name=f"wsp{c}") for c in range(2)]
        # cumulative-sum output; col 0 stays 0 (page -1 sample)
        scano = [state.tile([128, 257], F32, name=f"scano{c}") for c in range(2)]
        ones = state.tile([128, 1], F32)
        rcp = [state.tile([128, 1], F32, name=f"rcp{c}") for c in range(2)]
        # capture accumulators live on rows 64:128 (aligned with pt's END rows)
        capb = [state.tile([128, 1], F32, name=f"capb{c}") for c in range(2)]
        endbuf = [
            state.tile([128, S - (S // 2 - 1)], F32, name=f"endbuf{c}")
            for c in range(2)
        ]
        mxbuf = [state.tile([32, NR], F32, name=f"mxbuf{c}") for c in range(2)]
        hitx = [state.tile([128, S], F32, name=f"hitx{c}") for c in range(2)]
        hcum = state.tile([64, NR], F32)
        duppb = state.tile([128, 128], BF16)
        dupi = state.tile([128, 128], BF16)
        dupf = state.tile([128, 128], F32)
        tgacc = state.tile([128, 2 * len(chunks)], F32)
        gath = gathp.tile([128, out_blocks * 128], BF16)
        oh = gathp.tile([128, out_blocks * 128], BF16)
        gidx = gathp.tile([128, idx_cols], I16)

        nc.sync.dma_start(duppb[:], duppb_d[:])
        nc.sync.dma_start(dupi[:], dupi_d[:])
        for c in range(2):
            nc.sync.dma_start(hitx[c][:], hit_d[c][:])
        nc.sync.dma_start(hcum[:], hcum_d[:])
        nc.sync.dma_start(gidx[:], gidx_d[:])
        nc.sync.dma_start(oh[:], oh_d[:])

        for c in range(2):
            nc.vector.memset(capb[c][:], 0.0)
            nc.vector.memset(scano[c][:, 0:1], 0.0)
        nc.vector.memset(ones[:], 1.0)

        # ---- gather chunks (process over the whole scan) ----
        for (s0, ns, icol, ocol) in chunks:
            ni = ns * B
            src = scg_d[s0 : s0 + ns].rearrange("s b f t -> (s b f t)")
            src_blk = src.rearrange("(n e) -> n e", e=128)
            nc.gpsimd.dma_gather(
                gath[:, ocol * 128 : (ocol + ni // 128) * 128].rearrange(
                    "p (c e) -> p c e", e=128
                ),
                src_blk,
                gidx[:, icol : icol + ni // 16],
                num_idxs=ni,
                num_idxs_reg=ni,
                elem_size=128,
            )

        # ---- streamed score blocks + block exp ----
        def load_block(bi, c):
            s0 = bi * SBLK
            ns = min(SBLK, S - s0)
            blk = blkp[c].tile([128, SBLK * 256], BF16, tag="blk", name=f"blkt{c}")
            nc.sync.dma_start(
                blk[:, 0 : ns * 256], sc_d[c][:, s0 * 256 : (s0 + ns) * 256]
            )
            return blk

        blk = [load_block(0, 0), load_block(0, 1)]
        e16 = [None, None]
        for c in range(2):
            e16[c] = e16p[c].tile([128, SBLK * 256], BF16, tag="e16", name=f"e16t{c}")
            nc.scalar.activation(
                e16[c][:], blk[c][:], mybir.ActivationFunctionType.Exp
            )

        # ---- init from step 0: state0[b,t] = exp(sc[0, b, START_TAG, t]) ----
        # START_TAG=30 -> h'=1, j'=14; rhs cols = (s=0, tl, j'=14)
        pt = [None, None]
        for c in range(2):
            rhs0 = e16[c][:, 14:256:16]  # [128, 16] (tl strided)
            p0 = psum.tile([128, 16], F32, tag=f"pt{c}")
            nc.tensor.matmul(p0[:], dupi[:], rhs0, start=True, stop=True)
            pt[c] = p0

        # prefetch + exp next blocks
        blk_next = [load_block(1, 0), load_block(1, 1)]
        e16_next = [None, None]
        for c in range(2):
            e16_next[c] = e16p[c].tile(
                [128, SBLK * 256], BF16, tag="e16", name=f"e16t{c}"
            )
            nc.scalar.activation(
                e16_next[c][:], blk_next[c][:], mybir.ActivationFunctionType.Exp
            )

        # ---- main scan ----
        nren = [0, 0]
        last_scan_inst = None
        gchunk_emitted = 0

        def emit_capture(s):
            # emitted after both chains' main ops (PE mm already done); each
            # step writes its own endbuf column - no serial accumulate chain
            for c in range(2):
                nc.vector.tensor_copy(
                    endbuf[c][64:128, s - smin : s - smin + 1],
                    pt[c][64:128, 15:16],
                )

        for s in range(1, S):
            bi, sl = divmod(s, SBLK)
            if sl == 0:
                for c in range(2):
                    blk[c] = blk_next[c]
                    e16[c] = e16_next[c]
                if bi + 1 < nblk:
                    blk_next = [load_block(bi + 1, 0), load_block(bi + 1, 1)]

            # main chain ops (DVE: fused scan + sub, PE: mm); the state is
            # read straight from the previous step's PSUM tile - no copy hop
            for c in range(2):
                scale = rcp[c] if (s > 1 and (s - 1) % RENORM == 0 and (s - 1) <= S - 2) else ones
                ins = nc.vector._custom_dve(
                    MUL_CUMSUM_SCALE,
                    out=scano[c][:, 1:257],
                    in0=e16[c][:, sl * 256 : (sl + 1) * 256],
                    in1=pt[c][:].unsqueeze(1).broadcast_to([128, 16, 16]),
                    s0=scale[:],
                )
                last_scan_inst = ins
                seg = work.tile([128, 16], BF16, tag=f"seg{c}", name=f"segt{c}")
                nc.vector.tensor_sub(
                    seg[:], scano[c][:, 16:257:16], scano[c][:, 0:241:16]
                )
                ptc = psum.tile([128, 16], F32, tag=f"pt{c}")
                nc.tensor.matmul(ptc[:], duppb[:], seg[:], start=True, stop=True)
                pt[c] = ptc

            # renorm: scale = 1 / sum_t' state'[b, t'] (rows of pt are
            # per-b state copies, so a row-sum of pt gives the total); the
            # scale is applied by the ACT copy at THIS step (wsp <- pt*rcp),
            # so rcp here feeds the NEXT renorm... no: rcp must be ready
            # BEFORE this step's copy. Emit at s-1? No - we compute rcp at
            # the PREVIOUS step (s-1) from pt(s-1): state totals drift only
            # slowly, and the log bookkeeping uses the actual pt(s-1) sums,
            # applied to state(s): mathematically still exact because mxbuf
            # records exactly the factor applied. So: at renorm-trigger step
            # s (copy uses rcp computed below from pt(s-1)): AFTER the copy,
            # recompute rcp from pt(s) for the NEXT trigger.
            for c in range(2):
                if (s % RENORM) == 0 and s <= S - 2:
                    q = nren[c]
                    smr = work.tile([128, 1], F32, tag=f"smr{c}", name=f"smrt{c}")
                    nc.vector.reduce_sum(smr[:], pt[c][:], axis=mybir.AxisListType.X)
                    nc.vector.reciprocal(rcp[c][:], smr[:])
                    nc.vector.tensor_copy(mxbuf[c][:, q : q + 1], smr[0:32, :])
                    nren[c] += 1

            # capture for THIS step, emitted after both chains' main ops;
            # reads wsp (SBUF) which the ACT copy just produced
            if s >= smin:
                emit_capture(s)

            # exp for the NEXT block, split into 4-step halves spread over
            # this block's steps (keeps 1us-max stalls out of the ACT queue
            # at block boundaries)
            if bi + 1 < nblk:
                ns_next = min(SBLK, S - (bi + 1) * SBLK)
                if sl == 2:
                    for c in range(2):
                        e16_next[c] = e16p[c].tile(
                            [128, SBLK * 256], BF16, tag="e16", name=f"e16t{c}"
                        )
                if sl in (2, 3, 4, 5):
                    c = 0 if sl in (2, 3) else 1
                    half = 0 if sl in (2, 4) else 1
                    lo = half * 1024
                    hi = min((half + 1) * 1024, ns_next * 256)
                    if lo < hi:
                        nc.scalar.activation(
                            e16_next[c][:, lo:hi],
                            blk_next[c][:, lo:hi],
                            mybir.ActivationFunctionType.Exp,
                        )

            # tg masked-sum half-chunks on DVE, late in the scan
            if s >= 100 and s % 3 == 0 and gchunk_emitted < 2 * len(chunks):
                g, half = divmod(gchunk_emitted, 2)
                (s0g, nsg, icolg, ocolg) = chunks[g]
                ni = nsg * B
                ncols = (ni // 128) * 128
                lo = half * (ncols // 2)
                hi = ncols if half else ncols // 2
                if lo < hi:
                    tgtmp = work.tile([128, 512], BF16, tag="tgtmp")
                    nc.vector.scalar_tensor_tensor(
                        tgtmp[:, 0 : hi - lo],
                        gath[:, ocolg * 128 + lo : ocolg * 128 + hi],
                        1.0,
                        oh[:, ocolg * 128 + lo : ocolg * 128 + hi],
                        op0=mybir.AluOpType.mult,
                        op1=mybir.AluOpType.mult,
                        accum_out=tgacc[:, 2 * g + half : 2 * g + half + 1],
                    )
                gchunk_emitted += 1

        for c in range(2):
            assert nren[c] == NR, (nren[c], NR)

        # any remaining tg half-chunks
        while gchunk_emitted < 2 * len(chunks):
            g, half = divmod(gchunk_emitted, 2)
            (s0g, nsg, icolg, ocolg) = chunks[g]
            ni = nsg * B
            ncols = (ni // 128) * 128
            lo = half * (ncols // 2)
            hi = ncols if half else ncols // 2
            if lo < hi:
                tgtmp = work.tile([128, 512], BF16, tag="tgtmp")
                nc.vector.scalar_tensor_tensor(
                    tgtmp[:, 0 : hi - lo],
                    gath[:, ocolg * 128 + lo : ocolg * 128 + hi],
                    1.0,
                    oh[:, ocolg * 128 + lo : ocolg * 128 + hi],
                    op0=mybir.AluOpType.mult,
                    op1=mybir.AluOpType.mult,
                    accum_out=tgacc[:, 2 * g + half : 2 * g + half + 1],
                )
            gchunk_emitted += 1

        # ---- tg combine: per-partition totals then f32 dup matmul ----
        tgtot = state.tile([128, 1], F32)
        nc.vector.reduce_sum(tgtot[:], tgacc[:], axis=mybir.AxisListType.X)
        nc.sync.dma_start(dupf[:], dupf_d[:])
        ptg = psumg.tile([128, 1], F32, tag="tg")
        nc.tensor.matmul(ptg[:], dupf[:], tgtot[:], start=True, stop=True)

        # ---- capture masked-sum: capb = sum_s endbuf[:, s]*hit[:, s] ----
        for c in range(2):
            captmp = work.tile([128, 128], F32, tag="captmp", name=f"captmpt{c}")
            nc.vector.scalar_tensor_tensor(
                captmp[64:128, 0 : S - smin],
                endbuf[c][64:128, :],
                1.0,
                hitx[c][64:128, smin:S],
                op0=mybir.AluOpType.mult,
                op1=mybir.AluOpType.mult,
                accum_out=capb[c][64:128, :],
            )

        # ---- deferred logs + loss assembly ----
        # bring the capture accumulators (rows 64:96, th=0 copy) down to
        # base-0 rows via a tiny SBUF->SBUF DMA (engines cannot cross
        # partitions)
        cap_end = state.tile([64, 1], F32)
        for c in range(2):
            nc.sync.dma_start(cap_end[c * 32 : c * 32 + 32, :], capb[c][64:96, :])
        mxall = state.tile([64, NR], F32)
        for c in range(2):
            nc.sync.dma_start(mxall[c * 32 : c * 32 + 32, :], mxbuf[c][:])
        lnmx = state.tile([64, NR], F32)
        nc.scalar.activation(lnmx[:], mxall[:], mybir.ActivationFunctionType.Ln)
        capCtmp = state.tile([64, NR], F32)
        cap_C = state.tile([64, 1], F32)
        nc.vector.scalar_tensor_tensor(
            capCtmp[:],
            lnmx[:],
            1.0,
            hcum[:],
            op0=mybir.AluOpType.mult,
            op1=mybir.AluOpType.mult,
            accum_out=cap_C[:],
        )
        lw = state.tile([64, 1], F32)
        nc.scalar.activation(lw[:], cap_end[:], mybir.ActivationFunctionType.Ln)
        res = state.tile([64, 1], F32)
        nc.vector.tensor_add(res[:], cap_C[:], lw[:])
        nc.vector.tensor_sub(res[:], res[:], ptg[0:64, :])
        nc.sync.dma_start(out_d[:], res[:])

    nc.compile()
    return nc


def host_prep(scores_a: np.ndarray, targets_a: np.ndarray, mask: np.ndarray, S=S):
    """Per-annotator tensors: relayouted bf16 scores + index machinery."""
    chunks, idx_cols, out_blocks = _plan(S)
    NR = _n_renorms(S)

    # scan relayout: [c, h, th, b32, S, tl, j] (j innermost)
    x = scores_a.reshape(S, 2, BC, 2, 16, 2, 16)  # s, c, b, h, j, th, tl
    arr = np.ascontiguousarray(x.transpose(1, 3, 5, 2, 0, 6, 4)).astype(
        ml_dtypes.bfloat16
    )
    sc = arr.reshape(NCHAIN, 128, S * 256)
    scg = scores_a.astype(ml_dtypes.bfloat16)  # s-major gather copy

    tgt = targets_a.astype(np.int64)  # (S, B)
    maskf = mask.astype(np.float32)  # (S, B)
    lens = mask.astype(np.int64).sum(axis=0)  # (B,)
    assert lens.min() >= S // 2, "kernel assumes valid-prefix lens >= S//2"

    # hitx[c, 64 + th*32 + b_local, s] = 1 at b's cutoff step (rows 64:128
    # align with pt's END rows; duplicated over th)
    hitx = np.zeros((NCHAIN, 128, S), dtype=np.float32)
    hcum = np.zeros((64, NR), dtype=np.float32)
    for b in range(B):
        sb = int(lens[b]) - 1
        c, bl = divmod(b, BC)
        hitx[c, 64 + bl, sb] = 1.0
        hitx[c, 96 + bl, sb] = 1.0
        win = (sb - 1) // RENORM
        hcum[b, : min(win, NR)] = 1.0

    gidx = np.zeros((128, idx_cols), dtype=np.int16)
    oh = np.zeros((128, out_blocks * 128), dtype=ml_dtypes.bfloat16)
    ohv = oh.reshape(128, out_blocks, 128)
    for (s0, ns, icol, ocol) in chunks:
        ni = ns * B
        i = np.arange(ni)
        sl, bb = np.divmod(i, B)
        rel = (sl * B + bb) * (T * T) + tgt[s0 + sl, bb]
        blk, e = np.divmod(rel, 128)
        gidx[i % 16, icol + i // 16] = blk.astype(np.int16)
        ohv[i % 128, ocol + i // 128, e] = maskf[s0 + sl, bb]
    for g in range(1, 8):
        gidx[16 * g : 16 * (g + 1)] = gidx[:16]

    # dup matrices: p = (h'', th'', b'), po = (h, th, b)
    p = np.arange(128)
    po = np.arange(128)
    hpp, thpp, bpp = p // 64, (p // 32) % 2, p % 32
    hpo, thpo, bpo = po // 64, (po // 32) % 2, po % 32
    sel = (bpp[:, None] == bpo[None, :]) & (thpp[:, None] == hpo[None, :])
    duppb = sel.astype(ml_dtypes.bfloat16)
    dupi = (sel & (hpp[:, None] == 1)).astype(ml_dtypes.bfloat16)
    dupf = (p[:, None] % 64 == po[None, :] % 64).astype(np.float32)

    return dict(
        sc=sc, scg=scg, gidx=gidx, oh=oh, hit=hitx, hcum=hcum,
        duppb=duppb, dupi=dupi, dupf=dupf,
    )


_NC_CACHE = {}

TRACE = False
TRACE_DIR = None
LAST_RESULTS = None


def _get_nc(S=S):
    if S not in _NC_CACHE:
        _NC_CACHE[S] = build_nc(S)
    return _NC_CACHE[S]


def kernel(scores, targets, mask, a_mask):
    scores = np.asarray(scores)
    targets = np.asarray(targets)
    mask_np = np.asarray(mask).astype(bool)
    a_mask_np = np.asarray(a_mask).astype(bool)

    nc = _get_nc(scores.shape[1])

    in_maps = []
    for a in range(A):
        in_maps.append(host_prep(scores[a], targets[a], mask_np, S=scores.shape[1]))

    if TRACE:
        import antenv

        shim = "/opt/trn_rl_repo/antenv"
        if shim not in list(antenv.__path__):
            antenv.__path__.append(shim)

    global LAST_RESULTS
    res = run_bass_kernel_spmd(
        nc, in_maps, core_ids=list(range(A)), trace=TRACE, tmpdir=TRACE_DIR
    )
    LAST_RESULTS = res
    losses = np.stack([r["losses"][:, 0] for r in res.results])  # (A, B)
    loss = np.where(a_mask_np, losses, 0.0).sum(dtype=np.float32) / np.float32(B)
    return np.float32(loss)


# revision 32
# speedup vs baseline: 1.0503x; 1.0503x over previous
"""CRF loss (multi-annotator) Trainium2 kernel.

Problem (hardcoded): scores (8,200,64,32,32) f32, targets (8,200,64) int,
mask (200,64) bool, a_mask (8,64) bool -> scalar f32 loss.

Sharding: one annotator per NeuronCore (8 cores). Host applies a_mask and
sums / B.

Design:
  - Host relayouts scores to bf16 [c2, h2, th2, b32, S, tl16, j16]
    (f = h*16+j from-tag, t = th*16+tl to-tag, b-halves c as 2 independent
    scan chains interleaved for latency hiding). Per-partition-row DRAM
    spans are contiguous, so the stream DMA moves 4KB packets and half
    the bytes of the f32 original.
  - exp on ACT one 8-step block at a time, split into 1K-element pieces
    spread across the block's steps (amortizes the 224-cycle ACT bubble
    and keeps multi-us ops out of the ACT queue).
  - Per scan step and chain: ONE custom DVE op (MUL_CUMSUM_SCALE,
    registered at import: running sum of in0*in1*s0 along the free dim)
    fuses the e*w multiply, the renorm scale, and the cumulative sum.
    Segmented sums over j fall out as differences of page-end samples
    (tiny f32 tensor_sub -> bf16 seg), and ONE bf16 matmul both combines
    the f-halves across partitions AND lands the result directly in the
    state layout (lhsT encodes the th''=h selection). The next step's
    scan reads the state straight from the PSUM tile - no copy hop, so
    the chain is scan -> sub -> matmul -> scan.
  - renorm every RENORM steps: scale = 1/rowsum(pt) (uniform per b since
    pt rows are per-b state copies); fed to the next scan's s0 slot; all
    logs deferred to one end-of-kernel pass via host-built hcum windows.
  - capture-at-cutoff for the valid-prefix mask: each step past the
    earliest cutoff copies pt[END] into its own endbuf column (cheap,
    dependency-free), and ONE masked-sum STT at the end reduces
    endbuf * hit into the per-batch capture.
  - tg energy: dma_gather of 256B blocks from a separate s-major bf16
    copy of scores (chunk-relative int16 indices), then masked-sum STT
    half-chunks on DVE late in the scan, combined with an f32 dup matmul.
"""

import os
import sys

import numpy as np

if os.path.isdir("/opt/trn_rl_repo"):
    sys.path.insert(0, "/opt/trn_rl_repo")

import ml_dtypes  # noqa: E402

import concourse.bass as bass  # noqa: E402
import concourse.tile as tile  # noqa: E402
from concourse import bacc, mybir  # noqa: E402
from concourse.bass_utils import run_bass_kernel_spmd  # noqa: E402

F32 = mybir.dt.float32
BF16 = mybir.dt.bfloat16
I16 = mybir.dt.int16

A, S, B, T = 8, 200, 64, 32
START_TAG, END_TAG = 30, 31
SBLK = 8      # steps per streamed DMA block
GBLK = 16     # steps per dma_gather chunk
RENORM = 8    # renorm period (steps)

BC = 32       # batch elements per chain
NCHAIN = 2

# ---------------------------------------------------------------------------
# Custom DVE op: out[k] = running_sum(in0*in1*s0) (inclusive, whole stream)
# ---------------------------------------------------------------------------


def _register_mul_cumsum():
    import concourse.dve_ops as dve_ops
    from concourse.dve_ops import OPS, DveOp, DveOpSpec
    from concourse.dve_spec import AluOp, Spec, Src0, Src1, C0, lower, scan

    name = "MUL_CUMSUM_SCALE"
    for op in OPS:
        if op.name == name:
            return op

    spec = Spec(
        body=scan(AluOp.ADD, Src0 * Src1 * C0),
        reference=lambda in0, in1, s0: np.cumsum(
            in0.astype(np.float32) * in1 * s0, axis=-1
        ),
    )
    row = dve_ops._CUSTOM_DVE_ROW_BASE + len(OPS)
    shas = {}
    for ver in ("v3", "v4"):
        shas[ver] = DveOpSpec(
            name=name, opcode=row, uops=lower(spec, ver=ver), rd1_en=True
        ).sha(ver)
    op = DveOp(name, spec, subdim=False, uops_sha=shas)
    OPS.append(op)
    dve_ops.CUSTOM_DVE_SPECS[name] = spec
    dve_ops._SUB_OPCODE_FOR_NAME[name] = row
    return op


MUL_CUMSUM_SCALE = _register_mul_cumsum()


def _plan(S):
    """Gather chunk plan: list of (s0, nsteps, idx_col0, out_col0)."""
    chunks = []
    s0 = 0
    idx_col = 0
    out_col = 0
    while s0 < S:
        ns = min(GBLK, S - s0)
        ni = ns * B
        assert ni % 128 == 0
        chunks.append((s0, ns, idx_col, out_col))
        idx_col += ni // 16
        out_col += ni // 128
        s0 += ns
    return chunks, idx_col, out_col


def _n_renorms(S):
    # renorm triggered after steps s = RENORM, 2*RENORM, ... <= S-2
    return max(0, (S - 2) // RENORM)


def build_nc(S=S):
    from contextlib import ExitStack

    chunks, idx_cols, out_blocks = _plan(S)
    NR = _n_renorms(S)
    smin = S // 2 - 1  # earliest possible hit step (lens >= S//2)
    nblk = (S + SBLK - 1) // SBLK
    ROWELEMS = S * 256  # per-partition-row elements in the relayout

    nc = bacc.Bacc("TRN2", target_bir_lowering=False, debug=False, num_devices=8)

    # streamed scan layout: [chain, 128 rows (h,th,b32), S*256]
    sc_d = nc.dram_tensor("sc", [NCHAIN, 128, ROWELEMS], BF16, kind="ExternalInput").ap()
    # s-major bf16 copy for the tg gather
    scg_d = nc.dram_tensor("scg", [S, B, T, T], BF16, kind="ExternalInput").ap()
    gidx_d = nc.dram_tensor("gidx", [128, idx_cols], I16, kind="ExternalInput").ap()
    oh_d = nc.dram_tensor("oh", [128, out_blocks * 128], BF16, kind="ExternalInput").ap()
    hit_d = nc.dram_tensor("hit", [NCHAIN, 128, S], F32, kind="ExternalInput").ap()
    hcum_d = nc.dram_tensor("hcum", [64, NR], F32, kind="ExternalInput").ap()
    duppb_d = nc.dram_tensor("duppb", [128, 128], BF16, kind="ExternalInput").ap()
    dupi_d = nc.dram_tensor("dupi", [128, 128], BF16, kind="ExternalInput").ap()
    dupf_d = nc.dram_tensor("dupf", [128, 128], F32, kind="ExternalInput").ap()
    out_d = nc.dram_tensor("losses", [64, 1], F32, kind="ExternalOutput").ap()

    with tile.TileContext(nc) as tc, ExitStack() as ctx:
        state = ctx.enter_context(tc.tile_pool(name="state", bufs=1))
        blkp = [
            ctx.enter_context(tc.tile_pool(name=f"blk{c}", bufs=3)) for c in range(2)
        ]
        e16p = [
            ctx.enter_context(tc.tile_pool(name=f"e16{c}", bufs=3)) for c in range(2)
        ]
        work = ctx.enter_context(tc.tile_pool(name="work", bufs=4))
        gathp = ctx.enter_context(tc.tile_pool(name="gath", bufs=1))
        psum = ctx.enter_context(tc.tile_pool(name="psum", bufs=3, space="PSUM"))
        psumg = ctx.enter_context(tc.tile_pool(name="psumg", bufs=1, space="PSUM"))

        # ---- persistent state ----
        wsp = [state.tile([128, 16], BF16, name=f"wsp{c}") for c in range(2)]
        # cumulative-sum output; col 0 stays 0 (page -1 sample)
        scano = [state.tile([128, 257], F32, name=f"scano{c}") for c in range(2)]
        ones = state.tile([128, 1], F32)
        rcp = [state.tile([128, 1], F32, name=f"rcp{c}") for c in range(2)]
        # capture accumulators live on rows 64:128 (aligned with pt's END rows)
        capb = [state.tile([128, 1], F32, name=f"capb{c}") for c in range(2)]
        endbuf = [
            state.tile([128, S - (S // 2 - 1)], F32, name=f"endbuf{c}")
            for c in range(2)
        ]
        mxbuf = [state.tile([32, NR], F32, name=f"mxbuf{c}") for c in range(2)]
        hitx = [state.tile([128, S], F32, name=f"hitx{c}") for c in range(2)]
        hcum = state.tile([64, NR], F32)
        duppb = state.tile([128, 128], BF16)
        dupi = state.tile([128, 128], BF16)
        dupf = state.tile([128, 128], F32)
        tgacc = state.tile([128, 2 * len(chunks)], F32)
        gath = gathp.tile([128, out_blocks * 128], BF16)
        oh = gathp.tile([128, out_blocks * 128], BF16)
        gidx = gathp.tile([128, idx_cols], I16)

        nc.sync.dma_start(duppb[:], duppb_d[:])
        nc.sync.dma_start(dupi[:], dupi_d[:])
        for c in range(2):
            nc.sync.dma_start(hitx[c][:], hit_d[c][:])
        nc.sync.dma_start(hcum[:], hcum_d[:])
        nc.sync.dma_start(gidx[:], gidx_d[:])

        for c in range(2):
            nc.vector.memset(capb[c][:], 0.0)
            nc.vector.memset(scano[c][:, 0:1], 0.0)
        nc.vector.memset(ones[:], 1.0)

        # ---- streamed score blocks + block exp ----
        def load_block(bi, c):
            s0 = bi * SBLK
            ns = min(SBLK, S - s0)
            blk = blkp[c].tile([128, SBLK * 256], BF16, tag="blk", name=f"blkt{c}")
            nc.sync.dma_start(
                blk[:, 0 : ns * 256], sc_d[c][:, s0 * 256 : (s0 + ns) * 256]
            )
            return blk

        blk = [load_block(0, 0), load_block(0, 1)]
        e16 = [None, None]
        for c in range(2):
            e16[c] = e16p[c].tile([128, SBLK * 256], BF16, tag="e16", name=f"e16t{c}")
            nc.scalar.activation(
                e16[c][:], blk[c][:], mybir.ActivationFunctionType.Exp
            )

        # ---- init from step 0: state0[b,t] = exp(sc[0, b, START_TAG, t]) ----
        # START_TAG=30 -> h'=1, j'=14; rhs cols = (s=0, tl, j'=14)
        pt = [None, None]
        for c in range(2):
            rhs0 = e16[c][:, 14:256:16]  # [128, 16] (tl strided)
            p0 = psum.tile([128, 16], F32, tag=f"pt{c}")
            nc.tensor.matmul(p0[:], dupi[:], rhs0, start=True, stop=True)
            pt[c] = p0

        # prefetch + exp next blocks
        blk_next = [load_block(1, 0), load_block(1, 1)]
        e16_next = [None, None]
        for c in range(2):
            e16_next[c] = e16p[c].tile(
                [128, SBLK * 256], BF16, tag="e16", name=f"e16t{c}"
            )
            nc.scalar.activation(
                e16_next[c][:], blk_next[c][:], mybir.ActivationFunctionType.Exp
            )

        # ---- gather chunks + oh mask: emitted AFTER the first score blocks
        # so the scan's startup DMA isn't starved (gathers have ~300us slack)
        for (s0, ns, icol, ocol) in chunks:
            ni = ns * B
            gsrc = scg_d[s0 : s0 + ns].rearrange("s b f t -> (s b f t)")
            gsrc_blk = gsrc.rearrange("(n e) -> n e", e=128)
            nc.gpsimd.dma_gather(
                gath[:, ocol * 128 : (ocol + ni // 128) * 128].rearrange(
                    "p (c e) -> p c e", e=128
                ),
                gsrc_blk,
                gidx[:, icol : icol + ni // 16],
                num_idxs=ni,
                num_idxs_reg=ni,
                elem_size=128,
            )
        nc.sync.dma_start(oh[:], oh_d[:])

        # ---- main scan ----
        nren = [0, 0]
        last_scan_inst = None
        gchunk_emitted = 0

        def emit_capture(s):
            # emitted after both chains' main ops (PE mm already done); each
            # step writes its own endbuf column - no serial accumulate chain
            for c in range(2):
                nc.vector.tensor_copy(
                    endbuf[c][64:128, s - smin : s - smin + 1],
                    pt[c][64:128, 15:16],
                )

        for s in range(1, S):
            bi, sl = divmod(s, SBLK)
            if sl == 0:
                for c in range(2):
                    blk[c] = blk_next[c]
                    e16[c] = e16_next[c]
                if bi + 1 < nblk:
                    blk_next = [load_block(bi + 1, 0), load_block(bi + 1, 1)]

            # main chain ops (DVE: fused scan + sub, PE: mm); the state is
            # read straight from the previous step's PSUM tile - no copy hop
            for c in range(2):
                scale = rcp[c] if (s > 1 and (s - 1) % RENORM == 0 and (s - 1) <= S - 2) else ones
                ins = nc.vector._custom_dve(
                    MUL_CUMSUM_SCALE,
                    out=scano[c][:, 1:257],
                    in0=e16[c][:, sl * 256 : (sl + 1) * 256],
                    in1=pt[c][:].unsqueeze(1).broadcast_to([128, 16, 16]),
                    s0=scale[:],
                )
                last_scan_inst = ins
                seg = work.tile([128, 16], BF16, tag=f"seg{c}", name=f"segt{c}")
                nc.vector.tensor_sub(
                    seg[:], scano[c][:, 16:257:16], scano[c][:, 0:241:16]
                )
                ptc = psum.tile([128, 16], F32, tag=f"pt{c}")
                nc.tensor.matmul(ptc[:], duppb[:], seg[:], start=True, stop=True)
                pt[c] = ptc

            # renorm: scale = 1 / sum_t' state'[b, t'] (rows of pt are
            # per-b state copies, so a row-sum of pt gives the total); the
            # scale is applied by the ACT copy at THIS step (wsp <- pt*rcp),
            # so rcp here feeds the NEXT renorm... no: rcp must be ready
            # BEFORE this step's copy. Emit at s-1? No - we compute rcp at
            # the PREVIOUS step (s-1) from pt(s-1): state totals drift only
            # slowly, and the log bookkeeping uses the actual pt(s-1) sums,
            # applied to state(s): mathematically still exact because mxbuf
            # records exactly the factor applied. So: at renorm-trigger step
            # s (copy uses rcp computed below from pt(s-1)): AFTER the copy,
            # recompute rcp from pt(s) for the NEXT trigger.
            for c in range(2):
                if (s % RENORM) == 0 and s <= S - 2:
                    q = nren[c]
                    smr = work.tile([128, 1], F32, tag=f"smr{c}", name=f"smrt{c}")
                    nc.vector.reduce_sum(smr[:], pt[c][:], axis=mybir.AxisListType.X)
                    nc.vector.reciprocal(rcp[c][:], smr[:])
                    nc.vector.tensor_copy(mxbuf[c][:, q : q + 1], smr[0:32, :])
                    nren[c] += 1

            # capture for THIS step, emitted after both chains' main ops;
            # reads wsp (SBUF) which the ACT copy just produced
            if s >= smin:
                emit_capture(s)

            # exp for the NEXT block, split into 4-step halves spread over
            # this block's steps (keeps 1us-max stalls out of the ACT queue
            # at block boundaries)
            if bi + 1 < nblk:
                ns_next = min(SBLK, S - (bi + 1) * SBLK)
                if sl == 2:
                    for c in range(2):
                        e16_next[c] = e16p[c].tile(
                            [128, SBLK * 256], BF16, tag="e16", name=f"e16t{c}"
                        )
                if sl in (2, 3, 4, 5):
                    c = 0 if sl in (2, 3) else 1
                    half = 0 if sl in (2, 4) else 1
                    lo = half * 1024
                    hi = min((half + 1) * 1024, ns_next * 256)
                    if lo < hi:
                        nc.scalar.activation(
                            e16_next[c][:, lo:hi],
                            blk_next[c][:, lo:hi],
                            mybir.ActivationFunctionType.Exp,
                        )

            # tg masked-sum half-chunks on DVE, late in the scan
            if s >= 100 and s % 3 == 0 and gchunk_emitted < 2 * len(chunks):
                g, half = divmod(gchunk_emitted, 2)
                (s0g, nsg, icolg, ocolg) = chunks[g]
                ni = nsg * B
                ncols = (ni // 128) * 128
                lo = half * (ncols // 2)
                hi = ncols if half else ncols // 2
                if lo < hi:
                    tgtmp = work.tile([128, 512], BF16, tag="tgtmp")
                    nc.vector.scalar_tensor_tensor(
                        tgtmp[:, 0 : hi - lo],
                        gath[:, ocolg * 128 + lo : ocolg * 128 + hi],
                        1.0,
                        oh[:, ocolg * 128 + lo : ocolg * 128 + hi],
                        op0=mybir.AluOpType.mult,
                        op1=mybir.AluOpType.mult,
                        accum_out=tgacc[:, 2 * g + half : 2 * g + half + 1],
                    )
                gchunk_emitted += 1

        for c in range(2):
            assert nren[c] == NR, (nren[c], NR)

        # any remaining tg half-chunks
        while gchunk_emitted < 2 * len(chunks):
            g, half = divmod(gchunk_emitted, 2)
            (s0g, nsg, icolg, ocolg) = chunks[g]
            ni = nsg * B
            ncols = (ni // 128) * 128
            lo = half * (ncols // 2)
            hi = ncols if half else ncols // 2
            if lo < hi:
                tgtmp = work.tile([128, 512], BF16, tag="tgtmp")
                nc.vector.scalar_tensor_tensor(
                    tgtmp[:, 0 : hi - lo],
                    gath[:, ocolg * 128 + lo : ocolg * 128 + hi],
                    1.0,
                    oh[:, ocolg * 128 + lo : ocolg * 128 + hi],
                    op0=mybir.AluOpType.mult,
                    op1=mybir.AluOpType.mult,
                    accum_out=tgacc[:, 2 * g + half : 2 * g + half + 1],
                )
            gchunk_emitted += 1

        # ---- tg combine: per-partition totals then f32 dup matmul ----
        tgtot = state.tile([128, 1], F32)
        nc.vector.reduce_sum(tgtot[:], tgacc[:], axis=mybir.AxisListType.X)
        nc.sync.dma_start(dupf[:], dupf_d[:])
        ptg = psumg.tile([128, 1], F32, tag="tg")
        nc.tensor.matmul(ptg[:], dupf[:], tgtot[:], start=True, stop=True)

        # ---- capture masked-sum: capb = sum_s endbuf[:, s]*hit[:, s] ----
        for c in range(2):
            captmp = work.tile([128, 128], F32, tag="captmp", name=f"captmpt{c}")
            nc.vector.scalar_tensor_tensor(
                captmp[64:128, 0 : S - smin],
                endbuf[c][64:128, :],
                1.0,
                hitx[c][64:128, smin:S],
                op0=mybir.AluOpType.mult,
                op1=mybir.AluOpType.mult,
                accum_out=capb[c][64:128, :],
            )

        # ---- deferred logs + loss assembly ----
        # bring the capture accumulators (rows 64:96, th=0 copy) down to
        # base-0 rows via a tiny SBUF->SBUF DMA (engines cannot cross
        # partitions)
        cap_end = state.tile([64, 1], F32)
        for c in range(2):
            nc.sync.dma_start(cap_end[c * 32 : c * 32 + 32, :], capb[c][64:96, :])
        mxall = state.tile([64, NR], F32)
        for c in range(2):
            nc.sync.dma_start(mxall[c * 32 : c * 32 + 32, :], mxbuf[c][:])
        lnmx = state.tile([64, NR], F32)
        nc.scalar.activation(lnmx[:], mxall[:], mybir.ActivationFunctionType.Ln)
        capCtmp = state.tile([64, NR], F32)
        cap_C = state.tile([64, 1], F32)
        nc.vector.scalar_tensor_tensor(
            capCtmp[:],
            lnmx[:],
            1.0,
            hcum[:],
            op0=mybir.AluOpType.mult,
            op1=mybir.AluOpType.mult,
            accum_out=cap_C[:],
        )
        lw = state.tile([64, 1], F32)
        nc.scalar.activation(lw[:], cap_end[:], mybir.ActivationFunctionType.Ln)
        res = state.tile([64, 1], F32)
        nc.vector.tensor_add(res[:], cap_C[:], lw[:])
        nc.vector.tensor_sub(res[:], res[:], ptg[0:64, :])
        nc.sync.dma_start(out_d[:], res[:])

    nc.compile()
    return nc


def host_prep(scores_a: np.ndarray, targets_a: np.ndarray, mask: np.ndarray, S=S):
    """Per-annotator tensors: relayouted bf16 scores + index machinery."""
    chunks, idx_cols, out_blocks = _plan(S)
    NR = _n_renorms(S)

    # scan relayout: [c, h, th, b32, S, tl, j] (j innermost)
    x = scores_a.reshape(S, 2, BC, 2, 16, 2, 16)  # s, c, b, h, j, th, tl
    arr = np.ascontiguousarray(x.transpose(1, 3, 5, 2, 0, 6, 4)).astype(
        ml_dtypes.bfloat16
    )
    sc = arr.reshape(NCHAIN, 128, S * 256)
    scg = scores_a.astype(ml_dtypes.bfloat16)  # s-major gather copy

    tgt = targets_a.astype(np.int64)  # (S, B)
    maskf = mask.astype(np.float32)  # (S, B)
    lens = mask.astype(np.int64).sum(axis=0)  # (B,)
    assert lens.min() >= S // 2, "kernel assumes valid-prefix lens >= S//2"

    # hitx[c, 64 + th*32 + b_local, s] = 1 at b's cutoff step (rows 64:128
    # align with pt's END rows; duplicated over th)
    hitx = np.zeros((NCHAIN, 128, S), dtype=np.float32)
    hcum = np.zeros((64, NR), dtype=np.float32)
    for b in range(B):
        sb = int(lens[b]) - 1
        c, bl = divmod(b, BC)
        hitx[c, 64 + bl, sb] = 1.0
        hitx[c, 96 + bl, sb] = 1.0
        win = (sb - 1) // RENORM
        hcum[b, : min(win, NR)] = 1.0

    gidx = np.zeros((128, idx_cols), dtype=np.int16)
    oh = np.zeros((128, out_blocks * 128), dtype=ml_dtypes.bfloat16)
    ohv = oh.reshape(128, out_blocks, 128)
    for (s0, ns, icol, ocol) in chunks:
        ni = ns * B
        i = np.arange(ni)
        sl, bb = np.divmod(i, B)
        rel = (sl * B + bb) * (T * T) + tgt[s0 + sl, bb]
        blk, e = np.divmod(rel, 128)
        gidx[i % 16, icol + i // 16] = blk.astype(np.int16)
        ohv[i % 128, ocol + i // 128, e] = maskf[s0 + sl, bb]
    for g in range(1, 8):
        gidx[16 * g : 16 * (g + 1)] = gidx[:16]

    # dup matrices: p = (h'', th'', b'), po = (h, th, b)
    p = np.arange(128)
    po = np.arange(128)
    hpp, thpp, bpp = p // 64, (p // 32) % 2, p % 32
    hpo, thpo, bpo = po // 64, (po // 32) % 2, po % 32
    sel = (bpp[:, None] == bpo[None, :]) & (thpp[:, None] == hpo[None, :])
    duppb = sel.astype(ml_dtypes.bfloat16)
    dupi = (sel & (hpp[:, None] == 1)).astype(ml_dtypes.bfloat16)
    dupf = (p[:, None] % 64 == po[None, :] % 64).astype(np.float32)

    return dict(
        sc=sc, scg=scg, gidx=gidx, oh=oh, hit=hitx, hcum=hcum,
        duppb=duppb, dupi=dupi, dupf=dupf,
    )


_NC_CACHE = {}

TRACE = False
TRACE_DIR = None
LAST_RESULTS = None


def _get_nc(S=S):
    if S not in _NC_CACHE:
        _NC_CACHE[S] = build_nc(S)
    return _NC_CACHE[S]


def kernel(scores, targets, mask, a_mask):
    scores = np.asarray(scores)
    targets = np.asarray(targets)
    mask_np = np.asarray(mask).astype(bool)
    a_mask_np = np.asarray(a_mask).astype(bool)

    nc = _get_nc(scores.shape[1])

    in_maps = []
    for a in range(A):
        in_maps.append(host_prep(scores[a], targets[a], mask_np, S=scores.shape[1]))

    if TRACE:
        import antenv

        shim = "/opt/trn_rl_repo/antenv"
        if shim not in list(antenv.__path__):
            antenv.__path__.append(shim)

    global LAST_RESULTS
    res = run_bass_kernel_spmd(
        nc, in_maps, core_ids=list(range(A)), trace=TRACE, tmpdir=TRACE_DIR
    )
    LAST_RESULTS = res
    losses = np.stack([r["losses"][:, 0] for r in res.results])  # (A, B)
    loss = np.where(a_mask_np, losses, 0.0).sum(dtype=np.float32) / np.float32(B)
    return np.float32(loss)


# revision 33
# speedup vs baseline: 1.0518x; 1.0014x over previous
"""CRF loss (multi-annotator) Trainium2 kernel.

Problem (hardcoded): scores (8,200,64,32,32) f32, targets (8,200,64) int,
mask (200,64) bool, a_mask (8,64) bool -> scalar f32 loss.

Sharding: one annotator per NeuronCore (8 cores). Host applies a_mask and
sums / B.

Design:
  - Host relayouts scores to bf16 [c2, h2, th2, b32, S, tl16, j16]
    (f = h*16+j from-tag, t = th*16+tl to-tag, b-halves c as 2 independent
    scan chains interleaved for latency hiding). Per-partition-row DRAM
    spans are contiguous, so the stream DMA moves 4KB packets and half
    the bytes of the f32 original.
  - exp on ACT one 8-step block at a time, split into 1K-element pieces
    spread across the block's steps (amortizes the 224-cycle ACT bubble
    and keeps multi-us ops out of the ACT queue).
  - Per scan step and chain: ONE custom DVE op (MUL_CUMSUM_SCALE,
    registered at import: running sum of in0*in1*s0 along the free dim)
    fuses the e*w multiply, the renorm scale, and the cumulative sum.
    Segmented sums over j fall out as differences of page-end samples
    (tiny f32 tensor_sub -> bf16 seg), and ONE bf16 matmul both combines
    the f-halves across partitions AND lands the result directly in the
    state layout (lhsT encodes the th''=h selection). The next step's
    scan reads the state straight from the PSUM tile - no copy hop, so
    the chain is scan -> sub -> matmul -> scan.
  - renorm every RENORM steps: scale = 1/rowsum(pt) (uniform per b since
    pt rows are per-b state copies); fed to the next scan's s0 slot; all
    logs deferred to one end-of-kernel pass via host-built hcum windows.
  - capture-at-cutoff for the valid-prefix mask: each step past the
    earliest cutoff copies pt[END] into its own endbuf column (cheap,
    dependency-free), and ONE masked-sum STT at the end reduces
    endbuf * hit into the per-batch capture.
  - tg energy: dma_gather of 256B blocks from a separate s-major bf16
    copy of scores (chunk-relative int16 indices), then masked-sum STT
    half-chunks on DVE late in the scan, combined with an f32 dup matmul.
"""

import os
import sys

import numpy as np

if os.path.isdir("/opt/trn_rl_repo"):
    sys.path.insert(0, "/opt/trn_rl_repo")

import ml_dtypes  # noqa: E402

import concourse.bass as bass  # noqa: E402
import concourse.tile as tile  # noqa: E402
from concourse import bacc, mybir  # noqa: E402
from concourse.bass_utils import run_bass_kernel_spmd  # noqa: E402

F32 = mybir.dt.float32
BF16 = mybir.dt.bfloat16
I16 = mybir.dt.int16

A, S, B, T = 8, 200, 64, 32
START_TAG, END_TAG = 30, 31
SBLK = 8      # steps per streamed DMA block
GBLK = 16     # steps per dma_gather chunk
RENORM = 8    # renorm period (steps)

BC = 32       # batch elements per chain
NCHAIN = 2

# ---------------------------------------------------------------------------
# Custom DVE op: out[k] = running_sum(in0*in1*s0) (inclusive, whole stream)
# ---------------------------------------------------------------------------


def _register_mul_cumsum():
    import concourse.dve_ops as dve_ops
    from concourse.dve_ops import OPS, DveOp, DveOpSpec
    from concourse.dve_spec import AluOp, Spec, Src0, Src1, C0, lower, scan

    name = "MUL_CUMSUM_SCALE"
    for op in OPS:
        if op.name == name:
            return op

    spec = Spec(
        body=scan(AluOp.ADD, Src0 * Src1 * C0),
        reference=lambda in0, in1, s0: np.cumsum(
            in0.astype(np.float32) * in1 * s0, axis=-1
        ),
    )
    row = dve_ops._CUSTOM_DVE_ROW_BASE + len(OPS)
    shas = {}
    for ver in ("v3", "v4"):
        shas[ver] = DveOpSpec(
            name=name, opcode=row, uops=lower(spec, ver=ver), rd1_en=True
        ).sha(ver)
    op = DveOp(name, spec, subdim=False, uops_sha=shas)
    OPS.append(op)
    dve_ops.CUSTOM_DVE_SPECS[name] = spec
    dve_ops._SUB_OPCODE_FOR_NAME[name] = row
    return op


MUL_CUMSUM_SCALE = _register_mul_cumsum()


def _plan(S):
    """Gather chunk plan: list of (s0, nsteps, idx_col0, out_col0)."""
    chunks = []
    s0 = 0
    idx_col = 0
    out_col = 0
    while s0 < S:
        ns = min(GBLK, S - s0)
        ni = ns * B
        assert ni % 128 == 0
        chunks.append((s0, ns, idx_col, out_col))
        idx_col += ni // 16
        out_col += ni // 128
        s0 += ns
    return chunks, idx_col, out_col


def _n_renorms(S):
    # renorm triggered after steps s = RENORM, 2*RENORM, ... <= S-2
    return max(0, (S - 2) // RENORM)


def build_nc(S=S):
    from contextlib import ExitStack

    chunks, idx_cols, out_blocks = _plan(S)
    NR = _n_renorms(S)
    smin = S // 2 - 1  # earliest possible hit step (lens >= S//2)
    nblk = (S + SBLK - 1) // SBLK
    ROWELEMS = S * 256  # per-partition-row elements in the relayout

    nc = bacc.Bacc("TRN2", target_bir_lowering=False, debug=False, num_devices=8)

    # streamed scan layout: [chain, 128 rows (h,th,b32), S*256]
    sc_d = nc.dram_tensor("sc", [NCHAIN, 128, ROWELEMS], BF16, kind="ExternalInput").ap()
    # s-major bf16 copy for the tg gather
    scg_d = nc.dram_tensor("scg", [S, B, T, T], BF16, kind="ExternalInput").ap()
    gidx_d = nc.dram_tensor("gidx", [128, idx_cols], I16, kind="ExternalInput").ap()
    oh_d = nc.dram_tensor("oh", [128, out_blocks * 128], BF16, kind="ExternalInput").ap()
    hit_d = nc.dram_tensor("hit", [NCHAIN, 128, S], F32, kind="ExternalInput").ap()
    hcum_d = nc.dram_tensor("hcum", [64, NR], F32, kind="ExternalInput").ap()
    duppb_d = nc.dram_tensor("duppb", [128, 128], BF16, kind="ExternalInput").ap()
    dupi_d = nc.dram_tensor("dupi", [128, 128], BF16, kind="ExternalInput").ap()
    dupf_d = nc.dram_tensor("dupf", [128, 128], F32, kind="ExternalInput").ap()
    out_d = nc.dram_tensor("losses", [64, 1], F32, kind="ExternalOutput").ap()

    with tile.TileContext(nc) as tc, ExitStack() as ctx:
        state = ctx.enter_context(tc.tile_pool(name="state", bufs=1))
        blkp = [
            ctx.enter_context(tc.tile_pool(name=f"blk{c}", bufs=4)) for c in range(2)
        ]
        e16p = [
            ctx.enter_context(tc.tile_pool(name=f"e16{c}", bufs=4)) for c in range(2)
        ]
        work = ctx.enter_context(tc.tile_pool(name="work", bufs=4))
        gathp = ctx.enter_context(tc.tile_pool(name="gath", bufs=1))
        psum = ctx.enter_context(tc.tile_pool(name="psum", bufs=3, space="PSUM"))
        psumg = ctx.enter_context(tc.tile_pool(name="psumg", bufs=1, space="PSUM"))

        # ---- persistent state ----
        wsp = [state.tile([128, 16], BF16, name=f"wsp{c}") for c in range(2)]
        # cumulative-sum output; col 0 stays 0 (page -1 sample)
        scano = [state.tile([128, 257], F32, name=f"scano{c}") for c in range(2)]
        ones = state.tile([128, 1], F32)
        rcp = [state.tile([128, 1], F32, name=f"rcp{c}") for c in range(2)]
        # capture accumulators live on rows 64:128 (aligned with pt's END rows)
        capb = [state.tile([128, 1], F32, name=f"capb{c}") for c in range(2)]
        endbuf = [
            state.tile([128, S - (S // 2 - 1)], F32, name=f"endbuf{c}")
            for c in range(2)
        ]
        mxbuf = [state.tile([32, NR], F32, name=f"mxbuf{c}") for c in range(2)]
        hitx = [state.tile([128, S], F32, name=f"hitx{c}") for c in range(2)]
        hcum = state.tile([64, NR], F32)
        duppb = state.tile([128, 128], BF16)
        dupi = state.tile([128, 128], BF16)
        dupf = state.tile([128, 128], F32)
        tgacc = state.tile([128, 2 * len(chunks)], F32)
        gath = gathp.tile([128, out_blocks * 128], BF16)
        oh = gathp.tile([128, out_blocks * 128], BF16)
        gidx = gathp.tile([128, idx_cols], I16)

        nc.sync.dma_start(duppb[:], duppb_d[:])
        nc.sync.dma_start(dupi[:], dupi_d[:])
        for c in range(2):
            nc.sync.dma_start(hitx[c][:], hit_d[c][:])
        nc.sync.dma_start(hcum[:], hcum_d[:])
        nc.sync.dma_start(gidx[:], gidx_d[:])

        for c in range(2):
            nc.vector.memset(capb[c][:], 0.0)
            nc.vector.memset(scano[c][:, 0:1], 0.0)
        nc.vector.memset(ones[:], 1.0)

        # ---- streamed score blocks + block exp ----
        def load_block(bi, c):
            s0 = bi * SBLK
            ns = min(SBLK, S - s0)
            blk = blkp[c].tile([128, SBLK * 256], BF16, tag="blk", name=f"blkt{c}")
            nc.sync.dma_start(
                blk[:, 0 : ns * 256], sc_d[c][:, s0 * 256 : (s0 + ns) * 256]
            )
            return blk

        blk = [load_block(0, 0), load_block(0, 1)]
        e16 = [None, None]
        for c in range(2):
            e16[c] = e16p[c].tile([128, SBLK * 256], BF16, tag="e16", name=f"e16t{c}")
            nc.scalar.activation(
                e16[c][:], blk[c][:], mybir.ActivationFunctionType.Exp
            )

        # ---- init from step 0: state0[b,t] = exp(sc[0, b, START_TAG, t]) ----
        # START_TAG=30 -> h'=1, j'=14; rhs cols = (s=0, tl, j'=14)
        pt = [None, None]
        for c in range(2):
            rhs0 = e16[c][:, 14:256:16]  # [128, 16] (tl strided)
            p0 = psum.tile([128, 16], F32, tag=f"pt{c}")
            nc.tensor.matmul(p0[:], dupi[:], rhs0, start=True, stop=True)
            pt[c] = p0

        # prefetch + exp next blocks
        blk_next = [load_block(1, 0), load_block(1, 1)]
        e16_next = [None, None]
        for c in range(2):
            e16_next[c] = e16p[c].tile(
                [128, SBLK * 256], BF16, tag="e16", name=f"e16t{c}"
            )
            nc.scalar.activation(
                e16_next[c][:], blk_next[c][:], mybir.ActivationFunctionType.Exp
            )

        nc.sync.dma_start(oh[:], oh_d[:])
        gather_emitted = 0

        # ---- main scan ----
        nren = [0, 0]
        last_scan_inst = None
        gchunk_emitted = 0

        def emit_capture(s):
            # emitted after both chains' main ops (PE mm already done); each
            # step writes its own endbuf column - no serial accumulate chain
            for c in range(2):
                nc.vector.tensor_copy(
                    endbuf[c][64:128, s - smin : s - smin + 1],
                    pt[c][64:128, 15:16],
                )

        for s in range(1, S):
            bi, sl = divmod(s, SBLK)
            if sl == 0:
                for c in range(2):
                    blk[c] = blk_next[c]
                    e16[c] = e16_next[c]
                if bi + 1 < nblk:
                    blk_next = [load_block(bi + 1, 0), load_block(bi + 1, 1)]

            # main chain ops (DVE: fused scan + sub, PE: mm); the state is
            # read straight from the previous step's PSUM tile - no copy hop
            for c in range(2):
                scale = rcp[c] if (s > 1 and (s - 1) % RENORM == 0 and (s - 1) <= S - 2) else ones
                ins = nc.vector._custom_dve(
                    MUL_CUMSUM_SCALE,
                    out=scano[c][:, 1:257],
                    in0=e16[c][:, sl * 256 : (sl + 1) * 256],
                    in1=pt[c][:].unsqueeze(1).broadcast_to([128, 16, 16]),
                    s0=scale[:],
                )
                last_scan_inst = ins
                seg = work.tile([128, 16], BF16, tag=f"seg{c}", name=f"segt{c}")
                nc.vector.tensor_sub(
                    seg[:], scano[c][:, 16:257:16], scano[c][:, 0:241:16]
                )
                ptc = psum.tile([128, 16], F32, tag=f"pt{c}")
                nc.tensor.matmul(ptc[:], duppb[:], seg[:], start=True, stop=True)
                pt[c] = ptc

            # renorm: scale = 1 / sum_t' state'[b, t'] (rows of pt are
            # per-b state copies, so a row-sum of pt gives the total); the
            # scale is applied by the ACT copy at THIS step (wsp <- pt*rcp),
            # so rcp here feeds the NEXT renorm... no: rcp must be ready
            # BEFORE this step's copy. Emit at s-1? No - we compute rcp at
            # the PREVIOUS step (s-1) from pt(s-1): state totals drift only
            # slowly, and the log bookkeeping uses the actual pt(s-1) sums,
            # applied to state(s): mathematically still exact because mxbuf
            # records exactly the factor applied. So: at renorm-trigger step
            # s (copy uses rcp computed below from pt(s-1)): AFTER the copy,
            # recompute rcp from pt(s) for the NEXT trigger.
            for c in range(2):
                if (s % RENORM) == 0 and s <= S - 2:
                    q = nren[c]
                    smr = work.tile([128, 1], F32, tag=f"smr{c}", name=f"smrt{c}")
                    nc.vector.reduce_sum(smr[:], pt[c][:], axis=mybir.AxisListType.X)
                    nc.vector.reciprocal(rcp[c][:], smr[:])
                    nc.vector.tensor_copy(mxbuf[c][:, q : q + 1], smr[0:32, :])
                    nren[c] += 1

            # capture for THIS step, emitted after both chains' main ops;
            # reads wsp (SBUF) which the ACT copy just produced
            if s >= smin:
                emit_capture(s)

            # exp for the NEXT block, split into 4-step halves spread over
            # this block's steps (keeps 1us-max stalls out of the ACT queue
            # at block boundaries)
            if bi + 1 < nblk:
                ns_next = min(SBLK, S - (bi + 1) * SBLK)
                if sl == 2:
                    for c in range(2):
                        e16_next[c] = e16p[c].tile(
                            [128, SBLK * 256], BF16, tag="e16", name=f"e16t{c}"
                        )
                if sl in (2, 3, 4, 5):
                    c = 0 if sl in (2, 3) else 1
                    half = 0 if sl in (2, 4) else 1
                    lo = half * 1024
                    hi = min((half + 1) * 1024, ns_next * 256)
                    if lo < hi:
                        nc.scalar.activation(
                            e16_next[c][:, lo:hi],
                            blk_next[c][:, lo:hi],
                            mybir.ActivationFunctionType.Exp,
                        )

            # one gather chunk per 8 steps: spreads its 256B-packet burst
            if s % 8 == 3 and gather_emitted < len(chunks):
                (s0g, nsg, icolg, ocolg) = chunks[gather_emitted]
                nig = nsg * B
                gsrc = scg_d[s0g : s0g + nsg].rearrange("s b f t -> (s b f t)")
                gsrc_blk = gsrc.rearrange("(n e) -> n e", e=128)
                nc.gpsimd.dma_gather(
                    gath[:, ocolg * 128 : (ocolg + nig // 128) * 128].rearrange(
                        "p (c e) -> p c e", e=128
                    ),
                    gsrc_blk,
                    gidx[:, icolg : icolg + nig // 16],
                    num_idxs=nig,
                    num_idxs_reg=nig,
                    elem_size=128,
                )
                gather_emitted += 1

            # tg masked-sum half-chunks on DVE, late in the scan
            if s >= 100 and s % 3 == 0 and gchunk_emitted < 2 * len(chunks):
                g, half = divmod(gchunk_emitted, 2)
                (s0g, nsg, icolg, ocolg) = chunks[g]
                ni = nsg * B
                ncols = (ni // 128) * 128
                lo = half * (ncols // 2)
                hi = ncols if half else ncols // 2
                if lo < hi:
                    tgtmp = work.tile([128, 512], BF16, tag="tgtmp")
                    nc.vector.scalar_tensor_tensor(
                        tgtmp[:, 0 : hi - lo],
                        gath[:, ocolg * 128 + lo : ocolg * 128 + hi],
                        1.0,
                        oh[:, ocolg * 128 + lo : ocolg * 128 + hi],
                        op0=mybir.AluOpType.mult,
                        op1=mybir.AluOpType.mult,
                        accum_out=tgacc[:, 2 * g + half : 2 * g + half + 1],
                    )
                gchunk_emitted += 1

        for c in range(2):
            assert nren[c] == NR, (nren[c], NR)

        # any remaining tg half-chunks
        while gchunk_emitted < 2 * len(chunks):
            g, half = divmod(gchunk_emitted, 2)
            (s0g, nsg, icolg, ocolg) = chunks[g]
            ni = nsg * B
            ncols = (ni // 128) * 128
            lo = half * (ncols // 2)
            hi = ncols if half else ncols // 2
            if lo < hi:
                tgtmp = work.tile([128, 512], BF16, tag="tgtmp")
                nc.vector.scalar_tensor_tensor(
                    tgtmp[:, 0 : hi - lo],
                    gath[:, ocolg * 128 + lo : ocolg * 128 + hi],
                    1.0,
                    oh[:, ocolg * 128 + lo : ocolg * 128 + hi],
                    op0=mybir.AluOpType.mult,
                    op1=mybir.AluOpType.mult,
                    accum_out=tgacc[:, 2 * g + half : 2 * g + half + 1],
                )
            gchunk_emitted += 1

        # ---- tg combine: per-partition totals then f32 dup matmul ----
        tgtot = state.tile([128, 1], F32)
        nc.vector.reduce_sum(tgtot[:], tgacc[:], axis=mybir.AxisListType.X)
        nc.sync.dma_start(dupf[:], dupf_d[:])
        ptg = psumg.tile([128, 1], F32, tag="tg")
        nc.tensor.matmul(ptg[:], dupf[:], tgtot[:], start=True, stop=True)

        # ---- capture masked-sum: capb = sum_s endbuf[:, s]*hit[:, s] ----
        for c in range(2):
            captmp = work.tile([128, 128], F32, tag="captmp", name=f"captmpt{c}")
            nc.vector.scalar_tensor_tensor(
                captmp[64:128, 0 : S - smin],
                endbuf[c][64:128, :],
                1.0,
                hitx[c][64:128, smin:S],
                op0=mybir.AluOpType.mult,
                op1=mybir.AluOpType.mult,
                accum_out=capb[c][64:128, :],
            )

        # ---- deferred logs + loss assembly ----
        # bring the capture accumulators (rows 64:96, th=0 copy) down to
        # base-0 rows via a tiny SBUF->SBUF DMA (engines cannot cross
        # partitions)
        cap_end = state.tile([64, 1], F32)
        for c in range(2):
            nc.sync.dma_start(cap_end[c * 32 : c * 32 + 32, :], capb[c][64:96, :])
        mxall = state.tile([64, NR], F32)
        for c in range(2):
            nc.sync.dma_start(mxall[c * 32 : c * 32 + 32, :], mxbuf[c][:])
        lnmx = state.tile([64, NR], F32)
        nc.scalar.activation(lnmx[:], mxall[:], mybir.ActivationFunctionType.Ln)
        capCtmp = state.tile([64, NR], F32)
        cap_C = state.tile([64, 1], F32)
        nc.vector.scalar_tensor_tensor(
            capCtmp[:],
            lnmx[:],
            1.0,
            hcum[:],
            op0=mybir.AluOpType.mult,
            op1=mybir.AluOpType.mult,
            accum_out=cap_C[:],
        )
        lw = state.tile([64, 1], F32)
        nc.scalar.activation(lw[:], cap_end[:], mybir.ActivationFunctionType.Ln)
        res = state.tile([64, 1], F32)
        nc.vector.tensor_add(res[:], cap_C[:], lw[:])
        nc.vector.tensor_sub(res[:], res[:], ptg[0:64, :])
        nc.sync.dma_start(out_d[:], res[:])

    nc.compile()
    return nc


def host_prep(scores_a: np.ndarray, targets_a: np.ndarray, mask: np.ndarray, S=S):
    """Per-annotator tensors: relayouted bf16 scores + index machinery."""
    chunks, idx_cols, out_blocks = _plan(S)
    NR = _n_renorms(S)

    # scan relayout: [c, h, th, b32, S, tl, j] (j innermost)
    x = scores_a.reshape(S, 2, BC, 2, 16, 2, 16)  # s, c, b, h, j, th, tl
    arr = np.ascontiguousarray(x.transpose(1, 3, 5, 2, 0, 6, 4)).astype(
        ml_dtypes.bfloat16
    )
    sc = arr.reshape(NCHAIN, 128, S * 256)
    scg = scores_a.astype(ml_dtypes.bfloat16)  # s-major gather copy

    tgt = targets_a.astype(np.int64)  # (S, B)
    maskf = mask.astype(np.float32)  # (S, B)
    lens = mask.astype(np.int64).sum(axis=0)  # (B,)
    assert lens.min() >= S // 2, "kernel assumes valid-prefix lens >= S//2"

    # hitx[c, 64 + th*32 + b_local, s] = 1 at b's cutoff step (rows 64:128
    # align with pt's END rows; duplicated over th)
    hitx = np.zeros((NCHAIN, 128, S), dtype=np.float32)
    hcum = np.zeros((64, NR), dtype=np.float32)
    for b in range(B):
        sb = int(lens[b]) - 1
        c, bl = divmod(b, BC)
        hitx[c, 64 + bl, sb] = 1.0
        hitx[c, 96 + bl, sb] = 1.0
        win = (sb - 1) // RENORM
        hcum[b, : min(win, NR)] = 1.0

    gidx = np.zeros((128, idx_cols), dtype=np.int16)
    oh = np.zeros((128, out_blocks * 128), dtype=ml_dtypes.bfloat16)
    ohv = oh.reshape(128, out_blocks, 128)
    for (s0, ns, icol, ocol) in chunks:
        ni = ns * B
        i = np.arange(ni)
        sl, bb = np.divmod(i, B)
        rel = (sl * B + bb) * (T * T) + tgt[s0 + sl, bb]
        blk, e = np.divmod(rel, 128)
        gidx[i % 16, icol + i // 16] = blk.astype(np.int16)
        ohv[i % 128, ocol + i // 128, e] = maskf[s0 + sl, bb]
    for g in range(1, 8):
        gidx[16 * g : 16 * (g + 1)] = gidx[:16]

    # dup matrices: p = (h'', th'', b'), po = (h, th, b)
    p = np.arange(128)
    po = np.arange(128)
    hpp, thpp, bpp = p // 64, (p // 32) % 2, p % 32
    hpo, thpo, bpo = po // 64, (po // 32) % 2, po % 32
    sel = (bpp[:, None] == bpo[None, :]) & (thpp[:, None] == hpo[None, :])
    duppb = sel.astype(ml_dtypes.bfloat16)
    dupi = (sel & (hpp[:, None] == 1)).astype(ml_dtypes.bfloat16)
    dupf = (p[:, None] % 64 == po[None, :] % 64).astype(np.float32)

    return dict(
        sc=sc, scg=scg, gidx=gidx, oh=oh, hit=hitx, hcum=hcum,
        duppb=duppb, dupi=dupi, dupf=dupf,
    )


_NC_CACHE = {}

TRACE = False
TRACE_DIR = None
LAST_RESULTS = None


def _get_nc(S=S):
    if S not in _NC_CACHE:
        _NC_CACHE[S] = build_nc(S)
    return _NC_CACHE[S]


def kernel(scores, targets, mask, a_mask):
    scores = np.asarray(scores)
    targets = np.asarray(targets)
    mask_np = np.asarray(mask).astype(bool)
    a_mask_np = np.asarray(a_mask).astype(bool)

    nc = _get_nc(scores.shape[1])

    in_maps = []
    for a in range(A):
        in_maps.append(host_prep(scores[a], targets[a], mask_np, S=scores.shape[1]))

    if TRACE:
        import antenv

        shim = "/opt/trn_rl_repo/antenv"
        if shim not in list(antenv.__path__):
            antenv.__path__.append(shim)

    global LAST_RESULTS
    res = run_bass_kernel_spmd(
        nc, in_maps, core_ids=list(range(A)), trace=TRACE, tmpdir=TRACE_DIR
    )
    LAST_RESULTS = res
    losses = np.stack([r["losses"][:, 0] for r in res.results])  # (A, B)
    loss = np.where(a_mask_np, losses, 0.0).sum(dtype=np.float32) / np.float32(B)
    return np.float32(loss)
